# revision 1
# baseline (speedup 1.0000x reference)
"""Trainium2 Bass kernel for nn_Attention_54322746359846 (gnn_message_passing).

Math: the reference computes
    q, k, v = einsum('bd,sndh->sbnh', x, w_qkv)
    scores  = einsum('tnh,snh->tns', q/sqrt(Hd), k)
    masked  = einsum('ts,sna->tna', adj, scores)
    attn    = softmax(masked, axis=-1)
    head_w  = attn.sum(axis=(0, 2))          # == N exactly: softmax rows sum to 1
    y       = v * head_w[None, :, None]      # == N * v
    out     = y.reshape(N, -1) @ w_proj + b_proj

Every softmax row sums to 1 for any finite input, so head_w[h] == N (to float
epsilon) regardless of adj/q/k. The whole attention pipeline collapses to

    out = x @ (N * W_v @ w_proj) + b_proj,   W_v[d, h*Hd + j] = w_qkv[2, h, d, j]

which is a single [4096,512] @ [512,512] matmul. We fold the weight product on
the host (512^3 flops), shard the 4096 rows of x across the 8 NeuronCores, and
run the per-core [512,512] @ [512,512] matmul on the TensorEngine.

Per-core device kernel:
    inputs : xT [512, 512]  (the core's 512 rows of x, transposed so the
                             contraction dim lands on SBUF partitions)
             w  [512, 512]  (fused weight, replicated)
    output : out [512, 512] (the core's 512 output rows)
    16 accumulating matmuls (4 row-tiles x 4 k-chunks), PSUM -> SBUF -> DRAM.
"""

import numpy as np

import concourse.bass as bass
import concourse.bacc as bacc
import concourse.mybir as mybir
import concourse.tile as tile
from concourse.bass_utils import run_bass_kernel_spmd

N_CORES = 8
N_NODES = 4096
DIM = 512
ROWS = N_NODES // N_CORES  # 512 rows of x per core
P = 128                    # SBUF/PSUM partitions
NK = DIM // P              # 4 contraction chunks
NM = ROWS // P             # 4 output row tiles

_cache: dict = {}
last_result = None  # BassKernelResults of the most recent run (for test harness)


def _build_nc():
    nc = bacc.Bacc("TRN2")
    xT = nc.declare_dram_parameter("xT", [DIM, ROWS], mybir.dt.float32, isOutput=False)
    w = nc.declare_dram_parameter("w", [DIM, DIM], mybir.dt.float32, isOutput=False)
    out = nc.declare_dram_parameter("out", [ROWS, DIM], mybir.dt.float32, isOutput=True)

    with tile.TileContext(nc) as tc:
        with (
            tc.tile_pool(name="sbuf", bufs=1) as pool,
            tc.tile_pool(name="psum", bufs=1, space="PSUM") as psum_pool,
        ):
            x_tiles = []
            w_tiles = []
            for kc in range(NK):
                xt = pool.tile([P, ROWS], mybir.dt.float32, tag=f"x{kc}")
                nc.sync.dma_start(out=xt[:], in_=xT[kc * P : (kc + 1) * P, :])
                x_tiles.append(xt)
                wt = pool.tile([P, DIM], mybir.dt.float32, tag=f"w{kc}")
                nc.sync.dma_start(out=wt[:], in_=w[kc * P : (kc + 1) * P, :])
                w_tiles.append(wt)

            for m in range(NM):
                pt = psum_pool.tile([P, DIM], mybir.dt.float32, tag=f"p{m}")
                for kc in range(NK):
                    nc.tensor.matmul(
                        pt[:],
                        x_tiles[kc][:, m * P : (m + 1) * P],  # lhsT [k=128, m=128]
                        w_tiles[kc][:],                       # rhs  [k=128, n=512]
                        start=(kc == 0),
                        stop=(kc == NK - 1),
                    )
                ot = pool.tile([P, DIM], mybir.dt.float32, tag=f"o{m}")
                nc.vector.tensor_copy(ot[:], pt[:])
                nc.sync.dma_start(out=out[m * P : (m + 1) * P, :], in_=ot[:])

    nc.finalize()
    return nc


def kernel(x, adj, w_qkv, w_proj, b_proj):
    global last_result
    x = np.asarray(x, dtype=np.float32)
    w_qkv = np.asarray(w_qkv, dtype=np.float32)
    w_proj = np.asarray(w_proj, dtype=np.float32)
    b_proj = np.asarray(b_proj, dtype=np.float32)

    # Fold: W_v[d, h*Hd+j] = w_qkv[2, h, d, j]; W = (N * W_v) @ w_proj
    w_v = np.ascontiguousarray(w_qkv[2].transpose(1, 0, 2)).reshape(DIM, DIM)
    w_fused = (np.float32(N_NODES) * w_v) @ w_proj

    xT = np.ascontiguousarray(x.T)  # [DIM, N_NODES]

    if "nc" not in _cache:
        _cache["nc"] = _build_nc()
    nc = _cache["nc"]

    in_maps = [
        {
            "xT": np.ascontiguousarray(xT[:, c * ROWS : (c + 1) * ROWS]),
            "w": w_fused,
        }
        for c in range(N_CORES)
    ]
    res = run_bass_kernel_spmd(nc, in_maps, core_ids=list(range(N_CORES)))
    last_result = res
    out = np.concatenate([res.results[c]["out"] for c in range(N_CORES)], axis=0)
    return out + b_proj[None, :]
